# revision 2
# baseline (speedup 1.0000x reference)
"""LocalRNN (sliding-window GRU) Trainium2 Bass kernel — fp8 DoubleRow v2.

Problem: x:[8,2048,768] f32, GRU weights w_ih/w_hh:[768,2304], biases:[2304].
For every position t, run a ksize=8-step GRU over the window x[t-7..t]
(zero left-pad) and emit the final hidden state -> [8,2048,768].

v2 strategy (vs fp32r baseline):
  * Recurrent matmuls for steps 1-5 run in fp8(e4m3) DoubleRow perf mode:
    one PE instruction contracts TWO 128-row k-tiles at 0.5 cyc/row.
    GX for the r/z gates is stored as fp8 (hi, lo) PAIRS whose sum is
    ~fp16-accurate; the pair is added into PSUM by ONE DoubleRow matmul
    against a duplicated fp8 identity (replaces the fp16 identity-MM).
  * Steps 6-7 (the accuracy-critical tail) run in fp16 (1 cyc/row).
  * H master state is fp16 (2x DVE element-wise); an fp8 shadow copy h8
    is produced after steps 0-4 for the next fp8 step's matmuls.
  * Element-wise work is spread across DVE / ACT / Pool(gpsimd); the
    per-gate nonlinearities stay on ACT (sigmoid/tanh share one table).
Numerical sim of this exact scheme: rel err ~8e-3 (tolerance 2e-2).
"""

import sys
import time

import numpy as np

sys.path.insert(0, "/opt/trn_rl_repo")

import concourse.bass as bass  # noqa: E402
import concourse.tile as tile  # noqa: E402
from concourse import bacc, mybir  # noqa: E402
from concourse.masks import make_identity  # noqa: E402

F32 = mybir.dt.float32
F32R = mybir.dt.float32r
FP16 = mybir.dt.float16
FP8 = mybir.dt.float8e4
AF = mybir.ActivationFunctionType
OP = mybir.AluOpType
DR = mybir.MatmulPerfMode.DoubleRow

D = 768
G3 = 3 * D
KD = D // 128          # 6 k-tiles
M = G3 // 128          # 18 m-tiles (0-5 r, 6-11 z, 12-17 n)

# per-step matmul dtype for steps 1..KSIZE-1 ('f8' or 'f16'); index j-1
STEP_DTYPE = ["f8", "f8", "f8", "f8", "f8", "f16", "f16"]


def build(T=2048, KSIZE=8, CHUNK=512, repeat=1):
    NCH = T // CHUNK
    TP = T + KSIZE - 1
    SUB = CHUNK // 128

    nc = bacc.Bacc("TRN2", target_bir_lowering=False, debug=False)
    x = nc.dram_tensor("x", [T, D], F32, kind="ExternalInput").ap()
    w_ih = nc.dram_tensor("w_ih", [D, G3], F32R, kind="ExternalInput").ap()
    w_hh = nc.dram_tensor("w_hh", [D, G3], F32, kind="ExternalInput").ap()
    b_ih = nc.dram_tensor("b_ih", [G3], F32, kind="ExternalInput").ap()
    b_hh = nc.dram_tensor("b_hh", [G3], F32, kind="ExternalInput").ap()
    out = nc.dram_tensor("out", [T, D], F32, kind="ExternalOutput").ap()

    with tile.TileContext(nc) as tc:
        with tc.tile_pool(name="perm", bufs=1) as perm:
            ident_f = perm.tile([128, 128], F32, name="ident_f")
            make_identity(nc, ident_f[:])
            ident_h = perm.tile([128, 128], FP16, name="ident_h")
            nc.vector.tensor_copy(ident_h[:], ident_f[:])
            ident8 = perm.tile([128, 2, 128], FP8, name="ident8")
            nc.vector.tensor_copy(ident8[:, 0, :], ident_f[:])
            nc.vector.tensor_copy(ident8[:, 1, :], ident_f[:])

            bih_sb = perm.tile([128, M], F32, name="bih")
            nc.sync.dma_start(bih_sb[:], b_ih.rearrange("(m p) -> p m", p=128))
            bhh_sb = perm.tile([128, M], F32, name="bhh")
            nc.sync.dma_start(bhh_sb[:], b_hh.rearrange("(m p) -> p m", p=128))
            bsum = perm.tile([128, M], F32, name="bsum")
            nc.vector.tensor_tensor(bsum[:], bih_sb[:], bhh_sb[:], op=OP.add)

            whh8 = perm.tile([128, KD, G3], FP8, name="whh8")
            whh16 = perm.tile([128, KD, G3], FP16, name="whh16")

            # gx slabs: r/z as fp8 (hi,lo) pairs (bias b_ih+b_hh folded),
            # n as fp16 (b_ih folded)
            gxrz = perm.tile([128, 24, TP], FP8, name="gxrz")
            gxn = perm.tile([128, KD, TP], FP16, name="gxn")

            for rep in range(repeat):
                _emit_once(
                    nc, tc, rep, T, KSIZE, CHUNK, NCH, TP, SUB,
                    x, w_ih, w_hh, out,
                    ident_f, ident_h, ident8, bih_sb, bhh_sb, bsum,
                    whh8, whh16, gxrz, gxn,
                )

    nc.compile()
    return nc


def _emit_once(nc, tc, rep, T, KSIZE, CHUNK, NCH, TP, SUB,
               x, w_ih, w_hh, out,
               ident_f, ident_h, ident8, bih_sb, bhh_sb, bsum,
               whh8, whh16, gxrz, gxn):
    PAD = KSIZE - 1

    # ---------------- phase W: load + cast w_hh to fp16 and fp8 -----------
    with tc.tile_pool(name=f"wtmp{rep}", bufs=2) as wtp:
        for k in range(KD):
            wt = wtp.tile([128, G3], F32, name="wt")
            nc.sync.dma_start(wt[:], w_hh[k * 128:(k + 1) * 128, :])
            nc.scalar.activation(whh16[:, k, :], wt[:], AF.Copy)
            nc.vector.tensor_copy(whh8[:, k, :], wt[:])

        # ------------- phase 1: GX = w_ih.T @ X.T + biases ----------------
        with (
            tc.tile_pool(name=f"wih{rep}", bufs=1) as wihp,
            tc.tile_pool(name=f"xload{rep}", bufs=2) as xp,
            tc.tile_pool(name=f"xt{rep}", bufs=1) as xtp,
            tc.tile_pool(name=f"pad{rep}", bufs=1) as padp,
            tc.tile_pool(name=f"pst{rep}", bufs=2, space="PSUM") as ps_t,
            tc.tile_pool(name=f"psg{rep}", bufs=2, space="PSUM") as ps_g,
        ):
            wih_r = []
            for k in range(KD):
                w = wihp.tile([128, G3], F32R, name=f"wih{k}")
                nc.sync.dma_start(w[:], w_ih[k * 128:(k + 1) * 128, :])
                wih_r.append(w)

            # left-pad region: gx = bias only (zero input contribution)
            zt = padp.tile([128, PAD], F32, name="padzero")
            nc.vector.memset(zt[:], 0.0)
            for m in range(M):
                if m < 12:
                    hi = gxrz[:, 2 * m, 0:PAD]
                    nc.scalar.activation(hi, zt[:], AF.Identity,
                                         bias=bsum[:, m:m + 1])
                    nc.vector.scalar_tensor_tensor(
                        gxrz[:, 2 * m + 1, 0:PAD], zt[:], bsum[:, m:m + 1],
                        hi, op0=OP.add, op1=OP.subtract,
                    )
                else:
                    nc.scalar.activation(gxn[:, m - 12, 0:PAD], zt[:],
                                         AF.Identity, bias=bih_sb[:, m:m + 1])

            for c in range(NCH):
                xts = [xtp.tile([128, CHUNK], F32R, name=f"xt{k}")
                       for k in range(KD)]
                for i in range(SUB):
                    xn = xp.tile([128, D], F32, name="xn")
                    t0 = c * CHUNK + i * 128
                    nc.sync.dma_start(xn[:], x[t0:t0 + 128, :])
                    for k in range(KD):
                        pt = ps_t.tile([128, 128], F32, name="pt")
                        nc.tensor.transpose(
                            pt[:], xn[:, k * 128:(k + 1) * 128], ident_f[:])
                        nc.scalar.activation(
                            xts[k][:, i * 128:(i + 1) * 128], pt[:], AF.Copy)
                for m in range(M):
                    pg = ps_g.tile([128, CHUNK], F32, name="pg")
                    for k in range(KD):
                        nc.tensor.matmul(
                            pg[:], wih_r[k][:, m * 128:(m + 1) * 128], xts[k][:],
                            start=(k == 0), stop=(k == KD - 1),
                        )
                    ts = slice(PAD + c * CHUNK, PAD + (c + 1) * CHUNK)
                    if m < 12:
                        hi = gxrz[:, 2 * m, ts]
                        nc.scalar.activation(hi, pg[:], AF.Identity,
                                             bias=bsum[:, m:m + 1])
                        nc.vector.scalar_tensor_tensor(
                            gxrz[:, 2 * m + 1, ts], pg[:], bsum[:, m:m + 1],
                            hi, op0=OP.add, op1=OP.subtract,
                        )
                    else:
                        nc.scalar.activation(gxn[:, m - 12, ts], pg[:],
                                             AF.Identity,
                                             bias=bih_sb[:, m:m + 1])

    # ---------------- phase 2: the KSIZE GRU steps ------------------------
    with (
        tc.tile_pool(name=f"H{rep}", bufs=1) as hp,
        tc.tile_pool(name=f"tmp{rep}", bufs=3) as tp2,
        tc.tile_pool(name=f"ee{rep}", bufs=KD) as eep,
        tc.tile_pool(name=f"ost{rep}", bufs=2) as ostp,
        tc.tile_pool(name=f"ps2{rep}", bufs=2, space="PSUM") as ps2,
        tc.tile_pool(name=f"pso{rep}", bufs=2, space="PSUM") as ps_o,
    ):
        H = hp.tile([128, KD, T], FP16, name="H")
        h8 = hp.tile([128, KD, T], FP8, name="h8")

        for j in range(KSIZE):
            sdt = STEP_DTYPE[j - 1] if j >= 1 else None
            for c in range(NCH):
                cs = slice(c * CHUNK, (c + 1) * CHUNK)
                lo = j + c * CHUNK   # window into the padded gx time axis
                ls = slice(lo, lo + CHUNK)

                es = []
                for d in range(KD):
                    if j == 0:
                        # h0 = 0: pre-acts are GX alone; n needs r*b_hh_n
                        pre = tp2.tile([128, CHUNK], FP16, name="pre")
                        nc.vector.tensor_tensor(
                            pre[:], gxrz[:, 2 * d, ls],
                            gxrz[:, 2 * d + 1, ls], op=OP.add)
                        r = tp2.tile([128, CHUNK], FP16, name="r")
                        nc.scalar.activation(r[:], pre[:], AF.Sigmoid)
                        prez = tp2.tile([128, CHUNK], FP16, name="prez")
                        nc.vector.tensor_tensor(
                            prez[:], gxrz[:, 2 * (d + 6), ls],
                            gxrz[:, 2 * (d + 6) + 1, ls], op=OP.add)
                        zb = tp2.tile([128, CHUNK], FP16, name="zb")
                        nc.scalar.activation(zb[:], prez[:], AF.Sigmoid,
                                             scale=-1.0)
                        g1 = tp2.tile([128, CHUNK], FP16, name="g1")
                        nc.vector.scalar_tensor_tensor(
                            g1[:], r[:], bhh_sb[:, d + 12:d + 13],
                            gxn[:, d, ls], op0=OP.mult, op1=OP.add)
                        nc.scalar.activation(g1[:], g1[:], AF.Tanh)
                        nc.vector.tensor_tensor(H[:, d, cs], zb[:], g1[:],
                                                op=OP.mult)
                        nc.gpsimd.tensor_copy(h8[:, d, cs], H[:, d, cs])
                        continue

                    pr = ps2.tile([128, CHUNK], F32, name="pr")
                    pz = ps2.tile([128, CHUNK], F32, name="pz")
                    pn = ps2.tile([128, CHUNK], F32, name="pn")
                    if sdt == "f8":
                        for ps, m in ((pr, d), (pz, d + 6)):
                            for kk in (0, 2, 4):
                                nc.tensor.matmul(
                                    ps[:],
                                    whh8[:, kk:kk + 2,
                                         m * 128:(m + 1) * 128],
                                    h8[:, kk:kk + 2, cs],
                                    start=(kk == 0), stop=False,
                                    perf_mode=DR)
                            nc.tensor.matmul(
                                ps[:], ident8[:],
                                gxrz[:, 2 * m:2 * m + 2, ls],
                                start=False, stop=True, perf_mode=DR)
                        m = d + 12
                        for kk in (0, 2, 4):
                            nc.tensor.matmul(
                                pn[:],
                                whh8[:, kk:kk + 2, m * 128:(m + 1) * 128],
                                h8[:, kk:kk + 2, cs],
                                start=(kk == 0), stop=(kk == 4),
                                perf_mode=DR)
                    else:  # f16
                        for ps, m in ((pr, d), (pz, d + 6)):
                            for kk in range(KD):
                                nc.tensor.matmul(
                                    ps[:],
                                    whh16[:, kk, m * 128:(m + 1) * 128],
                                    H[:, kk, cs],
                                    start=(kk == 0), stop=False)
                            nc.tensor.matmul(
                                ps[:], ident8[:],
                                gxrz[:, 2 * m:2 * m + 2, ls],
                                start=False, stop=True, perf_mode=DR)
                        m = d + 12
                        for kk in range(KD):
                            nc.tensor.matmul(
                                pn[:], whh16[:, kk, m * 128:(m + 1) * 128],
                                H[:, kk, cs],
                                start=(kk == 0), stop=(kk == KD - 1))

                    r = tp2.tile([128, CHUNK], FP16, name="r")
                    nc.scalar.activation(r[:], pr[:], AF.Sigmoid)
                    zb = tp2.tile([128, CHUNK], FP16, name="zb")
                    nc.scalar.activation(zb[:], pz[:], AF.Sigmoid, scale=-1.0)
                    g1 = tp2.tile([128, CHUNK], FP16, name="g1")
                    nc.vector.scalar_tensor_tensor(
                        g1[:], pn[:], bhh_sb[:, d + 12:d + 13], r[:],
                        op0=OP.add, op1=OP.mult)
                    nc.vector.tensor_tensor(g1[:], g1[:], gxn[:, d, ls],
                                            op=OP.add)
                    nc.scalar.activation(g1[:], g1[:], AF.Tanh)  # g1 <- n
                    e = eep.tile([128, CHUNK], FP16, name="e")
                    nc.gpsimd.tensor_tensor(e[:], g1[:], H[:, d, cs],
                                            op=OP.subtract)
                    nc.vector.tensor_tensor(e[:], zb[:], e[:], op=OP.mult)
                    es.append((d, e))

                # deferred H update: this chunk's f16 matmuls read old H,
                # Pool's (n - H) reads old H
                for d, e in es:
                    nc.vector.tensor_tensor(H[:, d, cs], H[:, d, cs], e[:],
                                            op=OP.add)
                if 1 <= j <= 4:
                    # fp8 shadow for the next fp8 step (j+1 <= 5)
                    for d in range(KD):
                        nc.gpsimd.tensor_copy(h8[:, d, cs], H[:, d, cs])

                # ------------- phase 3: transpose H chunk -> out ----------
                if j == KSIZE - 1:
                    for i in range(SUB):
                        t0 = c * CHUNK + i * 128
                        og = ostp.tile([128, D], F32, name="og")
                        for dd in range(KD):
                            po = ps_o.tile([128, 128], FP16, name="po")
                            nc.tensor.transpose(
                                po[:], H[:, dd, t0:t0 + 128], ident_h[:])
                            nc.scalar.activation(
                                og[:, dd * 128:(dd + 1) * 128], po[:],
                                AF.Copy)
                        nc.sync.dma_start(out[t0:t0 + 128, :], og[:])


# --------------------------------------------------------------------------
# PJRT runner (resident buffers, jit built once)
# --------------------------------------------------------------------------
class BassRunner:
    def __init__(self, nc, n_cores: int):
        import jax
        from jax.sharding import Mesh, PartitionSpec
        from jax.experimental.shard_map import shard_map
        from concourse.bass2jax import (
            _bass_exec_p, install_neuronx_cc_hook, partition_id_tensor,
        )

        install_neuronx_cc_hook()
        self.jax = jax
        self.nc = nc
        self.n_cores = n_cores

        partition_name = (
            nc.partition_id_tensor.name if nc.partition_id_tensor else None
        )
        in_names, out_names, out_avals, zero_outs = [], [], [], []
        for alloc in nc.m.functions[0].allocations:
            if not isinstance(alloc, mybir.MemoryLocationSet):
                continue
            name = alloc.memorylocations[0].name
            if alloc.kind == "ExternalInput":
                if name != partition_name:
                    in_names.append(name)
            elif alloc.kind == "ExternalOutput":
                shape = tuple(alloc.tensor_shape)
                dtype = mybir.dt.np(alloc.dtype)
                out_names.append(name)
                out_avals.append(jax.core.ShapedArray(shape, dtype))
                zero_outs.append(np.zeros(shape, dtype))
        self.in_names = in_names
        self.out_names = out_names
        self.zero_outs = zero_outs
        n_params = len(in_names)
        all_in_names = list(in_names) + list(out_names)
        if partition_name is not None:
            all_in_names.append(partition_name)

        def _body(*args):
            operands = list(args)
            if partition_name is not None:
                operands.append(partition_id_tensor())
            outs = _bass_exec_p.bind(
                *operands,
                out_avals=tuple(out_avals),
                in_names=tuple(all_in_names),
                out_names=tuple(out_names),
                lowering_input_output_aliases=(),
                sim_require_finite=True,
                sim_require_nnan=True,
                nc=nc,
            )
            return tuple(outs)

        devices = jax.devices()[:n_cores]
        assert len(devices) == n_cores, (
            f"need {n_cores} neuron devices, have {len(jax.devices())}"
        )
        if n_cores == 1:
            self.fn = jax.jit(_body, keep_unused=True)
        else:
            mesh = Mesh(np.asarray(devices), ("core",))
            in_specs = (PartitionSpec("core"),) * (n_params + len(out_names))
            out_specs = (PartitionSpec("core"),) * len(out_names)
            self.fn = jax.jit(
                shard_map(_body, mesh=mesh, in_specs=in_specs,
                          out_specs=out_specs, check_rep=False),
                keep_unused=True,
            )
        self._dev_args = None

    def stage(self, in_maps):
        assert len(in_maps) == self.n_cores
        if self.n_cores == 1:
            concat = [np.asarray(in_maps[0][n]) for n in self.in_names]
            concat += list(self.zero_outs)
        else:
            concat = [
                np.concatenate([np.asarray(m[n]) for m in in_maps], axis=0)
                for n in self.in_names
            ]
            concat += [
                np.concatenate([z] * self.n_cores, axis=0)
                for z in self.zero_outs
            ]
        self._dev_args = self.jax.device_put(concat)
        self.jax.block_until_ready(self._dev_args)

    def run(self):
        outs = self.fn(*self._dev_args)
        self.jax.block_until_ready(outs)
        return outs

    def run_results(self):
        outs = self.run()
        per_core = [{} for _ in range(self.n_cores)]
        for name, arr in zip(self.out_names, outs):
            arr = np.asarray(arr)
            if self.n_cores == 1:
                per_core[0][name] = arr
            else:
                for c, s in enumerate(np.split(arr, self.n_cores, axis=0)):
                    per_core[c][name] = s
        return per_core

    def time_runs(self, iters=10, warmup=2):
        for _ in range(warmup):
            self.run()
        ts = []
        for _ in range(iters):
            t0 = time.perf_counter()
            self.run()
            ts.append(time.perf_counter() - t0)
        return ts


# --------------------------------------------------------------------------
# public entry point
# --------------------------------------------------------------------------
_CACHE = {}


def _get_runner(T, KSIZE, n_cores, repeat=1):
    key = (T, KSIZE, n_cores, repeat)
    if key not in _CACHE:
        nc = build(T=T, KSIZE=KSIZE, repeat=repeat)
        _CACHE[key] = BassRunner(nc, n_cores)
    return _CACHE[key]


def kernel(x, w_ih, w_hh, b_ih, b_hh, ksize):
    x = np.ascontiguousarray(np.asarray(x, dtype=np.float32))
    B, T, _D = x.shape
    ksize = int(ksize)
    runner = _get_runner(T, ksize, B)
    w_ih = np.ascontiguousarray(np.asarray(w_ih, dtype=np.float32))
    w_hh = np.ascontiguousarray(np.asarray(w_hh, dtype=np.float32))
    b_ih = np.ascontiguousarray(np.asarray(b_ih, dtype=np.float32))
    b_hh = np.ascontiguousarray(np.asarray(b_hh, dtype=np.float32))
    in_maps = [
        {"x": x[b], "w_ih": w_ih, "w_hh": w_hh, "b_ih": b_ih, "b_hh": b_hh}
        for b in range(B)
    ]
    runner.stage(in_maps)
    res = runner.run_results()
    return np.stack([res[b]["out"] for b in range(B)], axis=0)
